# revision 18
# baseline (speedup 1.0000x reference)
"""Trainium2 Bass kernel for nn_CCL_50740743635433 (class-collapsed CCL loss).

Math: with C=64 classes, pos_centroid[i] == class_centroid[labels[i]], so the
reference's 8192x8192 distance matrix collapses to 8192x64:
  class_sum[c,:]  = sum_{i: lab_i==c} preds[i,:]      (one-hot matmul)
  cent[c,:]       = class_sum[c,:] / count[c]
  sq[i,c]         = relu(|p_i|^2 + |cent_c|^2 - 2 p_i.cent_c)
  pos[i]          = sqrt(sq[i, lab_i]);  neg[i] = sqrt(min_{c != lab_i} sq[i,c])
  loss            = mean softplus(pos - neg + 0.2)

Distribution (8 cores, no collectives): every core receives the FULL preds and
computes the class sums redundantly (a cross-core collective costs ~70us on
this rig vs ~12us of local compute); each core then evaluates distances +
softplus only for its own 1024-row shard and returns a partial sum; the host
adds the 8 partials and divides by N.

Perf structure (measured on this rig; input streaming is HBM-contention bound
at ~2.3TB/s aggregate across the 8 cores, so overlap everything under it):
- Both big matmuls in bf16 (fp32 matmul is 4 cyc/row); verified numerically:
  the final loss moves ~3e-8 relative (errors wash out in the 8192-row mean).
- Phase A packs even/odd chunks into the two 64-column halves of the PE array
  (tile_position) so pairs of matmuls run concurrently; back-to-back matmuls
  pipeline at ~55ns each.
- Inputs stream in 8 x 512KB per-group tiles split across both HWDGE queues
  (a single shared tile creates false WAW deps that serialize the DMAs);
  f32->bf16 casts alternate vector/scalar engines, chasing the DMAs.
- iota comes from a gpsimd IOTA (a DMA'd constant would queue behind the
  megabyte preds transfers and stall the one-hot generation by ~5us).
- Class counts are matmul'd from the one-hot tiles during the input stream,
  so counts -> 1/count -> broadcast are off the post-phase-A critical path.
- |cent_c|^2 is folded into the Gram PSUM by a K=1 rank-1 matmul, so the
  per-chunk distance needs only Relu(psum + p^2_bias) on the scalar engine.
- sqrt via 2-iteration Newton rsqrt on the vector engine; the scalar engine
  stays on one activation-table family (dummy Ln/Exp ops at the start pull
  all ~1.3us table loads into the startup window).
"""

import sys

sys.path.insert(0, "/opt/trn_rl_repo")

import numpy as np

import concourse.bacc as bacc
import concourse.bass_utils as bass_utils
import concourse.mybir as mybir
import concourse.tile as tile

N = 8192
D = 128
C = 64
N_CORES = 8
ROWS_PER_CORE = N // N_CORES          # 1024
CHUNKS = N // 128                     # 64 chunks of 128 rows
OWN_CHUNKS = ROWS_PER_CORE // 128     # 8 chunks per core
GROUPS = 8
G = CHUNKS // GROUPS                  # 8 chunks per DMA group
ALPHA = 0.2
BIG = 1e10
HUGE = 1e20

f32 = mybir.dt.float32
bf16 = mybir.dt.bfloat16
i32 = mybir.dt.int32
Alu = mybir.AluOpType
Act = mybir.ActivationFunctionType
Ax = mybir.AxisListType

_compiled = None
last_results = None


def _build():
    import ml_dtypes

    nc = bacc.Bacc(
        "TRN2",
        target_bir_lowering=False,
        debug=False,
        enable_asserts=True,
        num_devices=N_CORES,
    )

    preds_d = nc.dram_tensor("preds", [N, D], f32, kind="ExternalInput")
    labels_d = nc.dram_tensor("labels", [128, CHUNKS], f32, kind="ExternalInput")
    mypreds_d = nc.dram_tensor("my_preds", [ROWS_PER_CORE, D], f32, kind="ExternalInput")
    mylab_d = nc.dram_tensor("my_labels", [128, OWN_CHUNKS], f32, kind="ExternalInput")
    out_d = nc.dram_tensor("out", [1, 1], f32, kind="ExternalOutput")

    # blob1 f32 [128, 257]: ident128 | ones_col | row0 of 129:257 = ones_row
    b1 = np.zeros((128, 257), dtype=np.float32)
    b1[:, 0:128] = np.eye(128, dtype=np.float32)
    b1[:, 128] = 1.0
    b1[0, 129:257] = 1.0
    blob1_d = nc.inline_tensor(b1, name="blob1")
    # blob2 bf16 [128, 258]: identb | row0 of 128:256 = ones_row | ones_col
    b2 = np.zeros((128, 258), dtype=ml_dtypes.bfloat16)
    b2[:, 0:128] = np.eye(128, dtype=ml_dtypes.bfloat16)
    b2[0, 128:256] = 1.0
    b2[:, 256] = 1.0
    blob2_d = nc.inline_tensor(b2, name="blob2")

    with tile.TileContext(nc) as tc:
        with (
            tc.tile_pool(name="cst", bufs=1) as cst,
            tc.tile_pool(name="big", bufs=1) as bigp,
            tc.tile_pool(name="wrk", bufs=1) as wrk,
            tc.tile_pool(name="scr", bufs=2) as scr,
            tc.tile_pool(name="pacc", bufs=1, space="PSUM") as pacc,
            tc.tile_pool(name="pt", bufs=2, space="PSUM") as pt,
            tc.tile_pool(name="pg", bufs=2, space="PSUM") as pg,
            tc.tile_pool(name="psm", bufs=2, space="PSUM") as psm,
        ):
            # ---- tiny inputs + on-device consts ----
            lsb = cst.tile([128, CHUNKS], f32)
            nc.sync.dma_start(lsb[:], labels_d.ap())
            mylsb = cst.tile([128, OWN_CHUNKS], f32)
            nc.sync.dma_start(mylsb[:], mylab_d.ap())
            iota_sb = cst.tile([128, C], f32)
            nc.gpsimd.iota(
                iota_sb[:], pattern=[[1, C]], base=0, channel_multiplier=0,
                allow_small_or_imprecise_dtypes=True,
            )
            iota_b = iota_sb[:].rearrange("p (j c) -> p j c", j=1)
            alpha_sb = cst.tile([128, 1], f32)
            nc.vector.memset(alpha_sb[:], ALPHA)

            # dummy Ln+Exp so activation-table loads happen at startup
            dmy = cst.tile([1, 1], f32)
            nc.scalar.activation(dmy[:], alpha_sb[0:1, :], Act.Ln, bias=1.0)
            nc.scalar.activation(dmy[:], dmy[:], Act.Exp, bias=alpha_sb[0:1, :])

            # one-hot per group (bf16), emitted first on the vector engine
            oh_g = []
            for q in range(GROUPS):
                t = bigp.tile([128, G, C], bf16, name=f"oh{q}", tag=f"oh{q}")
                nc.vector.tensor_tensor(
                    t[:],
                    lsb[:, q * G : (q + 1) * G].to_broadcast((128, G, C)),
                    iota_b.to_broadcast((128, G, C)),
                    Alu.is_equal,
                )
                oh_g.append(t)

            # ---- preds: 8 per-group tiles, DMAs on both HWDGE queues ----
            preds_re = preds_d.ap().rearrange("(j p) d -> p j d", p=128)
            psb_g, psbbf_g = [], []
            for g in range(GROUPS):
                pf = bigp.tile([128, G, D], f32, name=f"psb{g}", tag=f"psb{g}")
                pb = bigp.tile(
                    [128, G, D + 1], bf16, name=f"psbbf{g}", tag=f"psbbf{g}"
                )
                dma_eng = nc.sync if g % 2 == 0 else nc.scalar
                dma_eng.dma_start(pf[:], preds_re[:, g * G : (g + 1) * G, :])
                nc.vector.memset(pb[:, :, D : D + 1], 1.0)
                if g % 2 == 0:
                    nc.vector.tensor_copy(pb[:, :, 0:D], pf[:])
                else:
                    nc.scalar.activation(pb[:, :, 0:D], pf[:], Act.Copy)
                psb_g.append(pf)
                psbbf_g.append(pb)

            # own shard + const blobs (needed only post-phase-A)
            osb = wrk.tile([128, OWN_CHUNKS, D], f32)
            nc.scalar.dma_start(
                osb[:], mypreds_d.ap().rearrange("(j p) d -> p j d", p=128)
            )
            blob2 = cst.tile([128, 258], bf16)
            nc.sync.dma_start(blob2[:], blob2_d.ap())
            blob1 = cst.tile([128, 257], f32)
            nc.scalar.dma_start(blob1[:], blob1_d.ap())
            ident_sb = blob1[:, 0:128]
            onesc_sb = blob1[:, 128:129]
            onesr_sb = blob1[0:1, 129:257]
            identb_sb = blob2[:, 0:128]
            onesrb_sb = blob2[0:1, 128:256]
            onescb_sb = blob2[:, 256:257]

            # ---- class counts from one-hots (off the critical tail) ----
            # psum_cnt[0, c*G + j] = count of class c among group q chunk j,
            # accumulated over groups; then reduced over j on the DVE.
            psum_cnt = psm.tile([1, C * G], f32, name="psum_cnt", tag="sm")
            for q in range(GROUPS):
                nc.tensor.matmul(
                    psum_cnt[:],
                    onescb_sb,
                    oh_g[q][:].rearrange("p j c -> p c j"),
                    start=(q == 0),
                    stop=(q == GROUPS - 1),
                )
            cnt = wrk.tile([1, C], f32)
            nc.vector.tensor_reduce(
                cnt[:], psum_cnt[:].rearrange("p (c j) -> p c j", c=C),
                Ax.X, Alu.add,
            )
            safe = wrk.tile([1, C], f32)
            nc.vector.tensor_scalar(safe[:], cnt[:], 1.0, None, Alu.max)
            rrow = wrk.tile([1, C], f32)
            nc.vector.reciprocal(rrow[:], safe[:])
            ab_sb = wrk.tile([1, C], f32)
            nc.vector.tensor_scalar(
                ab_sb[:], cnt[:], 0.0, HUGE, Alu.is_equal, Alu.mult
            )
            psum_rb = psm.tile([128, C], f32, name="psum_rb", tag="sm")
            nc.tensor.matmul(psum_rb[:], onesr_sb, rrow[:])

            # ---- phase A: class sums, even/odd col-packed ----
            # psum_cs2[c, :] (c<64): sums over even chunks for class c
            # psum_cs2[64+c, :]:     sums over odd chunks for class c
            psum_cs2 = pacc.tile([128, D + 1], f32)
            for j in range(CHUNKS):
                g, jj = j // G, j % G
                half = j % 2
                nc.tensor.matmul(
                    psum_cs2[64 * half : 64 * half + 64, :],
                    oh_g[g][:, jj, :],
                    psbbf_g[g][:, jj, :],
                    start=(j < 2),
                    stop=(j >= CHUNKS - 2),
                    tile_position=(0, 64 * half),
                    skip_group_check=True,
                )

            # ---- own-shard prep (independent of phase A results) ----
            osb_bf = wrk.tile([128, OWN_CHUNKS, D], bf16)
            nc.vector.tensor_copy(osb_bf[:], osb[:])
            psq = wrk.tile([128, OWN_CHUNKS], f32)
            pts_bf = wrk.tile([128, OWN_CHUNKS, D], bf16)
            for j in range(OWN_CHUNKS):
                sqscr = scr.tile([128, D], f32, name=f"sqscr{j}", tag="sqscr")
                nc.scalar.activation(
                    sqscr[:], osb[:, j, :], Act.Square,
                    accum_out=psq[:, j : j + 1],
                )
                ptb = pt.tile([128, 128], bf16, name=f"ptb{j}", tag="ptb")
                nc.tensor.transpose(ptb[:], osb_bf[:, j, :], identb_sb)
                nc.scalar.activation(pts_bf[:, j, :], ptb[:], Act.Copy, scale=-2.0)

            # ---- centroids ----
            cs_sb = wrk.tile([128, D + 1], f32)
            nc.scalar.activation(cs_sb[:], psum_cs2[:], Act.Copy)
            # centT_bf[d, c] = (class_sum_even + class_sum_odd)[c,d] * recip[c]
            psum_ct = pt.tile([128, 128], f32, name="psum_ct", tag="ctp", bufs=1)
            nc.tensor.transpose(psum_ct[:], cs_sb[:, 0:D], ident_sb)
            ct_sb = wrk.tile([128, 128], f32)
            nc.scalar.activation(ct_sb[:], psum_ct[:], Act.Copy)
            ctsum = wrk.tile([128, C], f32)
            nc.vector.tensor_tensor(
                ctsum[:], ct_sb[:, 0:C], ct_sb[:, C : 2 * C], Alu.add
            )
            centT_bf = wrk.tile([128, C], bf16)
            nc.vector.tensor_tensor(
                centT_bf[:], ctsum[:], psum_rb[:], Alu.mult
            )

            # c_sq row (+1e20 on absent classes) in bf16 for the rank-1 fold
            sqc = wrk.tile([128, C], f32)
            nc.vector.tensor_tensor(sqc[:], centT_bf[:], centT_bf[:], Alu.mult)
            psum_csq = psm.tile([1, C], f32, name="psum_csq", tag="sm")
            nc.tensor.matmul(psum_csq[:], onesc_sb, sqc[:])
            csqr_bf = wrk.tile([1, C], bf16)
            nc.vector.tensor_tensor(
                csqr_bf[:], psum_csq[:], ab_sb[:], Alu.add
            )

            # own-chunk masks: ohinv[:, j, 0, :] = 1e10*onehot (neg mask),
            #                  ohinv[:, j, 1, :] = 1e10*(1-onehot) (pos mask)
            mk = wrk.tile([128, OWN_CHUNKS, C], f32)
            nc.vector.tensor_tensor(
                mk[:],
                mylsb[:].to_broadcast((128, OWN_CHUNKS, C)),
                iota_b.to_broadcast((128, OWN_CHUNKS, C)),
                Alu.is_equal,
            )
            ohinv = wrk.tile([128, OWN_CHUNKS, 2, C], f32)
            nc.vector.tensor_scalar(
                ohinv[:, :, 0, :], mk[:], BIG, None, Alu.mult
            )
            nc.vector.tensor_scalar(
                ohinv[:, :, 1, :], mk[:], -BIG, BIG, Alu.mult, Alu.add
            )

            # ---- phase F: per own chunk distances, masked mins ----
            # psum_g = -2*G + csq (rank-1 fold); sq = relu(psum_g + p^2)
            # pnsq even cols = negsq (min over other classes), odd = possq
            pnsq = wrk.tile([128, 2 * OWN_CHUNKS], f32)
            for j in range(OWN_CHUNKS):
                psum_g = pg.tile([128, C], f32, name=f"psum_g{j}", tag="g")
                nc.tensor.matmul(
                    psum_g[:], pts_bf[:, j, :], centT_bf[:],
                    start=True, stop=False,
                )
                nc.tensor.matmul(
                    psum_g[:], onesrb_sb, csqr_bf[:],
                    start=False, stop=True, skip_group_check=True,
                )
                sqj = scr.tile([128, C], f32, name=f"sqj{j}", tag="sqj")
                nc.scalar.activation(
                    sqj[:], psum_g[:], Act.Relu, bias=psq[:, j : j + 1]
                )
                pair = scr.tile([128, 2, C], f32, name=f"pair{j}", tag="pair")
                nc.vector.tensor_tensor(
                    pair[:],
                    sqj[:].rearrange("p (u c) -> p u c", u=1).to_broadcast(
                        (128, 2, C)
                    ),
                    ohinv[:, j, :, :],
                    Alu.add,
                )
                nc.vector.tensor_reduce(
                    pnsq[:, 2 * j : 2 * j + 2], pair[:], Ax.X, Alu.min
                )

            # ---- tail: sqrt via Newton rsqrt on DVE, then softplus ----
            W = 2 * OWN_CHUNKS
            z = wrk.tile([128, W], f32)
            tsh = wrk.tile([128, W], f32)
            nc.vector.tensor_scalar(
                tsh[:].bitcast(i32), pnsq[:].bitcast(i32), 1, None,
                Alu.logical_shift_right,
            )
            nc.vector.tensor_scalar(
                z[:].bitcast(i32), tsh[:].bitcast(i32), -1, 0x5F3759DF,
                Alu.mult, Alu.add,
            )
            t1 = wrk.tile([128, W], f32)
            for _ in range(2):
                nc.vector.tensor_tensor(t1[:], z[:], z[:], Alu.mult)
                nc.vector.tensor_tensor(t1[:], t1[:], pnsq[:], Alu.mult)
                nc.vector.tensor_scalar(
                    t1[:], t1[:], -0.5, 1.5, Alu.mult, Alu.add
                )
                nc.vector.tensor_tensor(z[:], z[:], t1[:], Alu.mult)
            pn = wrk.tile([128, W], f32)
            nc.vector.tensor_tensor(pn[:], pnsq[:], z[:], Alu.mult)

            # softplus(pos - neg + alpha) = ln(1 + exp(...))
            x = wrk.tile([128, OWN_CHUNKS], f32)
            nc.vector.tensor_tensor(
                x[:], pn[:, 1::2], pn[:, 0::2], Alu.subtract
            )
            e = wrk.tile([128, OWN_CHUNKS], f32)
            nc.scalar.activation(e[:], x[:], Act.Exp, bias=alpha_sb[:])
            sp = wrk.tile([128, OWN_CHUNKS], f32)
            nc.scalar.activation(sp[:], e[:], Act.Ln, bias=1.0)
            rowsum = wrk.tile([128, 1], f32)
            nc.vector.tensor_reduce(rowsum[:], sp[:], Ax.X, Alu.add)
            psum_out = psm.tile([1, 1], f32, name="psum_out", tag="sm")
            nc.tensor.matmul(psum_out[:], rowsum[:], onesc_sb)
            out_sb = wrk.tile([1, 1], f32)
            nc.scalar.activation(out_sb[:], psum_out[:], Act.Copy)
            nc.sync.dma_start(out_d.ap(), out_sb[:])

    nc.compile()
    return nc


def _get_compiled():
    global _compiled
    if _compiled is None:
        _compiled = _build()
    return _compiled


def _chunk_major_labels(lab_f32):
    # labels[j*128 + p] -> [p, j]
    n_chunks = lab_f32.shape[0] // 128
    return np.ascontiguousarray(lab_f32.reshape(n_chunks, 128).T)


def kernel(preds, labels, _trace=False):
    preds = np.ascontiguousarray(np.asarray(preds, dtype=np.float32))
    lab_f32 = np.asarray(labels, dtype=np.float32)
    assert preds.shape == (N, D) and lab_f32.shape == (N,)

    nc = _get_compiled()
    lab_cm = _chunk_major_labels(lab_f32)
    in_maps = []
    for c in range(N_CORES):
        r0, r1 = c * ROWS_PER_CORE, (c + 1) * ROWS_PER_CORE
        in_maps.append(
            {
                "preds": preds,
                "labels": lab_cm,
                "my_preds": np.ascontiguousarray(preds[r0:r1]),
                "my_labels": _chunk_major_labels(lab_f32[r0:r1]),
            }
        )

    res = bass_utils.run_bass_kernel_spmd(
        nc, in_maps, core_ids=list(range(N_CORES)), trace=_trace
    )
    global last_results
    last_results = res
    total = sum(float(res.results[c]["out"][0, 0]) for c in range(N_CORES))
    return np.float32(total / N)


# revision 19
# speedup vs baseline: 1.0445x; 1.0445x over previous
"""Trainium2 Bass kernel for nn_CCL_50740743635433 (class-collapsed CCL loss).

Math: with C=64 classes, pos_centroid[i] == class_centroid[labels[i]], so the
reference's 8192x8192 distance matrix collapses to 8192x64:
  class_sum[c,:]  = sum_{i: lab_i==c} preds[i,:]      (one-hot matmul)
  cent[c,:]       = class_sum[c,:] / count[c]
  sq[i,c]         = relu(|p_i|^2 + |cent_c|^2 - 2 p_i.cent_c)
  pos[i]          = sqrt(sq[i, lab_i]);  neg[i] = sqrt(min_{c != lab_i} sq[i,c])
  loss            = mean softplus(pos - neg + 0.2)

Distribution (8 cores, no collectives): every core receives the FULL preds and
computes the class sums redundantly (a cross-core collective costs ~70us on
this rig vs ~12us of local compute); each core then evaluates distances +
softplus only for its own 1024-row shard and returns a partial sum; the host
adds the 8 partials and divides by N.

Perf structure (measured on this rig; input streaming is HBM-contention bound
at ~2.3TB/s aggregate across the 8 cores, so overlap everything under it):
- Both big matmuls in bf16 (fp32 matmul is 4 cyc/row); verified numerically:
  the final loss moves ~3e-8 relative (errors wash out in the 8192-row mean).
- Phase A packs even/odd chunks into the two 64-column halves of the PE array
  (tile_position) so pairs of matmuls run concurrently; back-to-back matmuls
  pipeline at ~55ns each.
- Inputs stream in 8 x 512KB per-group tiles split across both HWDGE queues
  (a single shared tile creates false WAW deps that serialize the DMAs);
  f32->bf16 casts alternate vector/scalar engines, chasing the DMAs.
- iota comes from a gpsimd IOTA (a DMA'd constant would queue behind the
  megabyte preds transfers and stall the one-hot generation by ~5us).
- Class counts are matmul'd from the one-hot tiles during the input stream,
  so counts -> 1/count -> broadcast are off the post-phase-A critical path.
- |cent_c|^2 is folded into the Gram PSUM by a K=1 rank-1 matmul, so the
  per-chunk distance needs only Relu(psum + p^2_bias) on the scalar engine.
- sqrt via 2-iteration Newton rsqrt on the vector engine; the scalar engine
  stays on one activation-table family (dummy Ln/Exp ops at the start pull
  all ~1.3us table loads into the startup window).
"""

import sys

sys.path.insert(0, "/opt/trn_rl_repo")

import numpy as np

import concourse.bacc as bacc
import concourse.bass_utils as bass_utils
import concourse.mybir as mybir
import concourse.tile as tile

N = 8192
D = 128
C = 64
N_CORES = 8
ROWS_PER_CORE = N // N_CORES          # 1024
CHUNKS = N // 128                     # 64 chunks of 128 rows
OWN_CHUNKS = ROWS_PER_CORE // 128     # 8 chunks per core
GROUPS = 8
G = CHUNKS // GROUPS                  # 8 chunks per DMA group
ALPHA = 0.2
BIG = 1e10
HUGE = 1e20

f32 = mybir.dt.float32
bf16 = mybir.dt.bfloat16
i32 = mybir.dt.int32
Alu = mybir.AluOpType
Act = mybir.ActivationFunctionType
Ax = mybir.AxisListType

_compiled = None
last_results = None


def _build():
    import ml_dtypes

    nc = bacc.Bacc(
        "TRN2",
        target_bir_lowering=False,
        debug=False,
        enable_asserts=True,
        num_devices=N_CORES,
    )

    preds_d = nc.dram_tensor("preds", [N, D], f32, kind="ExternalInput")
    labels_d = nc.dram_tensor("labels", [128, CHUNKS], f32, kind="ExternalInput")
    mypreds_d = nc.dram_tensor("my_preds", [ROWS_PER_CORE, D], f32, kind="ExternalInput")
    mylab_d = nc.dram_tensor("my_labels", [128, OWN_CHUNKS], f32, kind="ExternalInput")
    out_d = nc.dram_tensor("out", [1, 1], f32, kind="ExternalOutput")

    # blob1 f32 [128, 129]: ones_col | row0 of 1:129 = ones_row
    b1 = np.zeros((128, 129), dtype=np.float32)
    b1[:, 0] = 1.0
    b1[0, 1:129] = 1.0
    blob1_d = nc.inline_tensor(b1, name="blob1")
    # blob2 bf16 [128, 258]: identb | row0 of 128:256 = ones_row | ones_col
    b2 = np.zeros((128, 258), dtype=ml_dtypes.bfloat16)
    b2[:, 0:128] = np.eye(128, dtype=ml_dtypes.bfloat16)
    b2[0, 128:256] = 1.0
    b2[:, 256] = 1.0
    blob2_d = nc.inline_tensor(b2, name="blob2")

    with tile.TileContext(nc) as tc:
        with (
            tc.tile_pool(name="cst", bufs=1) as cst,
            tc.tile_pool(name="big", bufs=1) as bigp,
            tc.tile_pool(name="wrk", bufs=1) as wrk,
            tc.tile_pool(name="scr", bufs=2) as scr,
            tc.tile_pool(name="pacc", bufs=1, space="PSUM") as pacc,
            tc.tile_pool(name="pt", bufs=2, space="PSUM") as pt,
            tc.tile_pool(name="pg", bufs=2, space="PSUM") as pg,
            tc.tile_pool(name="psm", bufs=2, space="PSUM") as psm,
        ):
            # ---- tiny inputs + on-device consts ----
            lsb = cst.tile([128, CHUNKS], f32)
            nc.sync.dma_start(lsb[:], labels_d.ap())
            mylsb = cst.tile([128, OWN_CHUNKS], f32)
            nc.sync.dma_start(mylsb[:], mylab_d.ap())
            iota_sb = cst.tile([128, C], f32)
            nc.gpsimd.iota(
                iota_sb[:], pattern=[[1, C]], base=0, channel_multiplier=0,
                allow_small_or_imprecise_dtypes=True,
            )
            iota_b = iota_sb[:].rearrange("p (j c) -> p j c", j=1)
            alpha_sb = cst.tile([128, 1], f32)
            nc.vector.memset(alpha_sb[:], ALPHA)

            # dummy Ln+Exp so activation-table loads happen at startup
            dmy = cst.tile([1, 1], f32)
            nc.scalar.activation(dmy[:], alpha_sb[0:1, :], Act.Ln, bias=1.0)
            nc.scalar.activation(dmy[:], dmy[:], Act.Exp, bias=alpha_sb[0:1, :])

            # one-hot per group (bf16), emitted first on the vector engine
            oh_g = []
            for q in range(GROUPS):
                t = bigp.tile([128, G, C], bf16, name=f"oh{q}", tag=f"oh{q}")
                nc.vector.tensor_tensor(
                    t[:],
                    lsb[:, q * G : (q + 1) * G].to_broadcast((128, G, C)),
                    iota_b.to_broadcast((128, G, C)),
                    Alu.is_equal,
                )
                oh_g.append(t)

            # ---- preds: 8 per-group tiles, DMAs on both HWDGE queues ----
            preds_re = preds_d.ap().rearrange("(j p) d -> p j d", p=128)
            psb_g, psbbf_g = [], []
            for g in range(GROUPS):
                pf = bigp.tile([128, G, D], f32, name=f"psb{g}", tag=f"psb{g}")
                pb = bigp.tile(
                    [128, G, D], bf16, name=f"psbbf{g}", tag=f"psbbf{g}"
                )
                dma_eng = nc.sync if g % 2 == 0 else nc.scalar
                dma_eng.dma_start(pf[:], preds_re[:, g * G : (g + 1) * G, :])
                if g % 2 == 0:
                    nc.vector.tensor_copy(pb[:], pf[:])
                else:
                    nc.scalar.activation(pb[:], pf[:], Act.Copy)
                psb_g.append(pf)
                psbbf_g.append(pb)

            # own shard + const blobs (needed only post-phase-A)
            osb = wrk.tile([128, OWN_CHUNKS, D], f32)
            nc.scalar.dma_start(
                osb[:], mypreds_d.ap().rearrange("(j p) d -> p j d", p=128)
            )
            blob2 = cst.tile([128, 258], bf16)
            nc.sync.dma_start(blob2[:], blob2_d.ap())
            blob1 = cst.tile([128, 129], f32)
            nc.scalar.dma_start(blob1[:], blob1_d.ap())
            onesc_sb = blob1[:, 0:1]
            onesr_sb = blob1[0:1, 1:129]
            identb_sb = blob2[:, 0:128]
            onesrb_sb = blob2[0:1, 128:256]
            onescb_sb = blob2[:, 256:257]

            # ---- class counts from one-hots (off the critical tail) ----
            # psum_cnt[0, c*G + j] = count of class c among group q chunk j,
            # accumulated over groups; then reduced over j on the DVE.
            psum_cnt = psm.tile([1, C * G], f32, name="psum_cnt", tag="sm")
            for q in range(GROUPS):
                nc.tensor.matmul(
                    psum_cnt[:],
                    onescb_sb,
                    oh_g[q][:].rearrange("p j c -> p c j"),
                    start=(q == 0),
                    stop=(q == GROUPS - 1),
                )
            cnt = wrk.tile([1, C], f32)
            nc.vector.tensor_reduce(
                cnt[:], psum_cnt[:].rearrange("p (c j) -> p c j", c=C),
                Ax.X, Alu.add,
            )
            safe = wrk.tile([1, C], f32)
            nc.vector.tensor_scalar(safe[:], cnt[:], 1.0, None, Alu.max)
            rrow = wrk.tile([1, C], f32)
            nc.vector.reciprocal(rrow[:], safe[:])
            ab_sb = wrk.tile([1, C], f32)
            nc.vector.tensor_scalar(
                ab_sb[:], cnt[:], 0.0, HUGE, Alu.is_equal, Alu.mult
            )
            psum_rb = psm.tile([128, C], f32, name="psum_rb", tag="sm")
            nc.tensor.matmul(psum_rb[:], onesr_sb, rrow[:])

            # ---- phase A: class sums, directly transposed ----
            # psum_csT[d, c] = sum_i preds[i, d] * onehot[i, c]
            psum_csT = pacc.tile([128, C], f32)
            for j in range(CHUNKS):
                g, jj = j // G, j % G
                nc.tensor.matmul(
                    psum_csT[:],
                    psbbf_g[g][:, jj, :],
                    oh_g[g][:, jj, :],
                    start=(j == 0),
                    stop=(j == CHUNKS - 1),
                )

            # ---- own-shard prep (independent of phase A results) ----
            osb_bf = wrk.tile([128, OWN_CHUNKS, D], bf16)
            nc.vector.tensor_copy(osb_bf[:], osb[:])
            psq = wrk.tile([128, OWN_CHUNKS], f32)
            pts_bf = wrk.tile([128, OWN_CHUNKS, D], bf16)
            for j in range(OWN_CHUNKS):
                sqscr = scr.tile([128, D], f32, name=f"sqscr{j}", tag="sqscr")
                nc.scalar.activation(
                    sqscr[:], osb[:, j, :], Act.Square,
                    accum_out=psq[:, j : j + 1],
                )
                ptb = pt.tile([128, 128], bf16, name=f"ptb{j}", tag="ptb")
                nc.tensor.transpose(ptb[:], osb_bf[:, j, :], identb_sb)
                nc.scalar.activation(pts_bf[:, j, :], ptb[:], Act.Copy, scale=-2.0)

            # ---- centroids: centT_bf[d, c] = class_sumT[d,c] * recip[c] ----
            ct_sb = wrk.tile([128, C], f32)
            nc.scalar.activation(ct_sb[:], psum_csT[:], Act.Copy)
            centT_bf = wrk.tile([128, C], bf16)
            nc.vector.tensor_tensor(
                centT_bf[:], ct_sb[:], psum_rb[:], Alu.mult
            )

            # c_sq row (+1e20 on absent classes) in bf16 for the rank-1 fold
            sqc = wrk.tile([128, C], f32)
            nc.vector.tensor_tensor(sqc[:], centT_bf[:], centT_bf[:], Alu.mult)
            psum_csq = psm.tile([1, C], f32, name="psum_csq", tag="sm")
            nc.tensor.matmul(psum_csq[:], onesc_sb, sqc[:])
            csqr_bf = wrk.tile([1, C], bf16)
            nc.vector.tensor_tensor(
                csqr_bf[:], psum_csq[:], ab_sb[:], Alu.add
            )

            # own-chunk masks: ohinv[:, j, 0, :] = 1e10*onehot (neg mask),
            #                  ohinv[:, j, 1, :] = 1e10*(1-onehot) (pos mask)
            mk = wrk.tile([128, OWN_CHUNKS, C], f32)
            nc.vector.tensor_tensor(
                mk[:],
                mylsb[:].to_broadcast((128, OWN_CHUNKS, C)),
                iota_b.to_broadcast((128, OWN_CHUNKS, C)),
                Alu.is_equal,
            )
            ohinv = wrk.tile([128, OWN_CHUNKS, 2, C], f32)
            nc.vector.tensor_scalar(
                ohinv[:, :, 0, :], mk[:], BIG, None, Alu.mult
            )
            nc.vector.tensor_scalar(
                ohinv[:, :, 1, :], mk[:], -BIG, BIG, Alu.mult, Alu.add
            )

            # ---- phase F: per own chunk distances, masked mins ----
            # psum_g = -2*G + csq (rank-1 fold); sq = relu(psum_g + p^2)
            # pnsq even cols = negsq (min over other classes), odd = possq
            pnsq = wrk.tile([128, 2 * OWN_CHUNKS], f32)
            for j in range(OWN_CHUNKS):
                psum_g = pg.tile([128, C], f32, name=f"psum_g{j}", tag="g")
                nc.tensor.matmul(
                    psum_g[:], pts_bf[:, j, :], centT_bf[:],
                    start=True, stop=False,
                )
                nc.tensor.matmul(
                    psum_g[:], onesrb_sb, csqr_bf[:],
                    start=False, stop=True, skip_group_check=True,
                )
                sqj = scr.tile([128, C], f32, name=f"sqj{j}", tag="sqj")
                nc.scalar.activation(
                    sqj[:], psum_g[:], Act.Relu, bias=psq[:, j : j + 1]
                )
                pair = scr.tile([128, 2, C], f32, name=f"pair{j}", tag="pair")
                nc.vector.tensor_tensor(
                    pair[:],
                    sqj[:].rearrange("p (u c) -> p u c", u=1).to_broadcast(
                        (128, 2, C)
                    ),
                    ohinv[:, j, :, :],
                    Alu.add,
                )
                nc.vector.tensor_reduce(
                    pnsq[:, 2 * j : 2 * j + 2], pair[:], Ax.X, Alu.min
                )

            # ---- tail: sqrt via Newton rsqrt on DVE, then softplus.
            # Processed in two halves so the first half's chain overlaps the
            # second half's phase-F work. ----
            W = 2 * OWN_CHUNKS
            z = wrk.tile([128, W], f32)
            tsh = wrk.tile([128, W], f32)
            t1 = wrk.tile([128, W], f32)
            pn = wrk.tile([128, W], f32)
            x = wrk.tile([128, OWN_CHUNKS], f32)
            e = wrk.tile([128, OWN_CHUNKS], f32)
            sp = wrk.tile([128, OWN_CHUNKS], f32)
            HW_ = W // 2
            HO = OWN_CHUNKS // 2
            for h in range(2):
                ps = pnsq[:, h * HW_ : (h + 1) * HW_]
                zs = z[:, h * HW_ : (h + 1) * HW_]
                ts_ = tsh[:, h * HW_ : (h + 1) * HW_]
                t1s = t1[:, h * HW_ : (h + 1) * HW_]
                pns = pn[:, h * HW_ : (h + 1) * HW_]
                nc.vector.tensor_scalar(
                    ts_.bitcast(i32), ps.bitcast(i32), 1, None,
                    Alu.logical_shift_right,
                )
                nc.vector.tensor_scalar(
                    zs.bitcast(i32), ts_.bitcast(i32), -1, 0x5F3759DF,
                    Alu.mult, Alu.add,
                )
                for _ in range(2):
                    nc.vector.tensor_tensor(t1s, zs, zs, Alu.mult)
                    nc.vector.tensor_tensor(t1s, t1s, ps, Alu.mult)
                    nc.vector.tensor_scalar(
                        t1s, t1s, -0.5, 1.5, Alu.mult, Alu.add
                    )
                    nc.vector.tensor_tensor(zs, zs, t1s, Alu.mult)
                nc.vector.tensor_tensor(pns, ps, zs, Alu.mult)
                # softplus(pos - neg + alpha) = ln(1 + exp(...))
                xs = x[:, h * HO : (h + 1) * HO]
                nc.vector.tensor_tensor(
                    xs, pns[:, 1::2], pns[:, 0::2], Alu.subtract
                )
                es = e[:, h * HO : (h + 1) * HO]
                nc.scalar.activation(es, xs, Act.Exp, bias=alpha_sb[:])
                nc.scalar.activation(
                    sp[:, h * HO : (h + 1) * HO], es, Act.Ln, bias=1.0
                )
            rowsum = wrk.tile([128, 1], f32)
            nc.vector.tensor_reduce(rowsum[:], sp[:], Ax.X, Alu.add)
            psum_out = psm.tile([1, 1], f32, name="psum_out", tag="sm")
            nc.tensor.matmul(psum_out[:], rowsum[:], onesc_sb)
            out_sb = wrk.tile([1, 1], f32)
            nc.scalar.activation(out_sb[:], psum_out[:], Act.Copy)
            nc.sync.dma_start(out_d.ap(), out_sb[:])

    nc.compile()
    return nc


def _get_compiled():
    global _compiled
    if _compiled is None:
        _compiled = _build()
    return _compiled


def _chunk_major_labels(lab_f32):
    # labels[j*128 + p] -> [p, j]
    n_chunks = lab_f32.shape[0] // 128
    return np.ascontiguousarray(lab_f32.reshape(n_chunks, 128).T)


def kernel(preds, labels, _trace=False):
    preds = np.ascontiguousarray(np.asarray(preds, dtype=np.float32))
    lab_f32 = np.asarray(labels, dtype=np.float32)
    assert preds.shape == (N, D) and lab_f32.shape == (N,)

    nc = _get_compiled()
    lab_cm = _chunk_major_labels(lab_f32)
    in_maps = []
    for c in range(N_CORES):
        r0, r1 = c * ROWS_PER_CORE, (c + 1) * ROWS_PER_CORE
        in_maps.append(
            {
                "preds": preds,
                "labels": lab_cm,
                "my_preds": np.ascontiguousarray(preds[r0:r1]),
                "my_labels": _chunk_major_labels(lab_f32[r0:r1]),
            }
        )

    res = bass_utils.run_bass_kernel_spmd(
        nc, in_maps, core_ids=list(range(N_CORES)), trace=_trace
    )
    global last_results
    last_results = res
    total = sum(float(res.results[c]["out"][0, 0]) for c in range(N_CORES))
    return np.float32(total / N)
